# revision 36
# baseline (speedup 1.0000x reference)
"""BERT self-attention block (QKV -> attention -> dense -> residual+LN) on 8 trn2 NeuronCores.

Sharding: data-parallel over batch (2) x tensor-parallel over heads (4 heads/core).
Compute body runs fp8 DoubleRow matmuls (QKV projection, attention ctx, dense) at
2x PE throughput with host-side scale folding (wqk*64, wv*32, wd*64, evac /2048),
and full-128-contraction score matmuls via zero-padded per-head Q tiles (the K
bias is dropped -- softmax is invariant to per-query constant shifts). Per-core
dense partials are summed with a chunked bf16 ReduceScatter over each batch
group ([[0,1,2,3],[4,5,6,7]]); each core finishes residual+LayerNorm on its own
token shard and the host reassembles the full [2, 2048, 1024] output.
"""

import sys

for _p in ("/opt/trn_rl_repo",):
    if _p not in sys.path:
        sys.path.insert(0, _p)

import numpy as np
import ml_dtypes

import concourse.bass as bass
import concourse.mybir as mybir
import concourse.tile as tile
from concourse import bacc
from concourse.bass_utils import run_bass_kernel_spmd

BF16 = ml_dtypes.bfloat16
FP8 = ml_dtypes.float8_e4m3  # TRN float8e4 (max +-240, inf at S.1111.000)

HIDDEN = 1024
HEADS = 16
HD = 64  # head dim
B = 2
S = 2048
LN_EPS = 1e-5

N_CORES = 8
TP = 4  # tensor-parallel ranks per batch group
LHEADS = HEADS // TP  # 4 local heads
PAIRS = LHEADS // 2  # 2 head pairs
NCP = 4  # contraction chunk-pairs (hidden 1024 = 4 x 256)
NTOK = S // 128  # 16 token chunks
NKP = NTOK // 2  # 8 k-chunk pairs for DoubleRow ctx
NQT = 4  # attention q-tiles (512 q each)
QT = S // NQT  # 512
REPLICA_GROUPS = [[0, 1, 2, 3], [4, 5, 6, 7]]
# ReduceScatter chunk boundaries in 128-token units
RS_CHUNKS = [(0, 4), (4, 8), (8, 12), (12, 16)]
NCHUNK = len(RS_CHUNKS)
# per-rank rows per chunk (chunk token count / 4 ranks)
RS_SZ = [(hi - lo) * 32 for lo, hi in RS_CHUNKS]
# padded layout: chunk g's rows live at [g*128, g*128+sz) in hs_res / out
PAD_ROWS = NCHUNK * 128

# host-side scale folding for fp8 operands
SW = 64.0  # wqk, bqk
SVS = 32.0  # wv
SD = 64.0  # wd
EXP_SCALE = 0.125 / (SW * SW)  # 2^-15, exact
SCC = 256.0  # fp8 collective payload scale (partials sigma~0.005 -> ~1.3)
DENSE_DESCALE = SCC / (SVS * SD)  # 2^-3, exact
CC_DESCALE = 1.0 / SCC  # 2^-8, exact

dt = mybir.dt
Alu = mybir.AluOpType
Act = mybir.ActivationFunctionType
PM = mybir.MatmulPerfMode


def _build_program():
    nc = bacc.Bacc(
        "TRN2", target_bir_lowering=False, debug=False, num_devices=N_CORES
    )

    # Route Exp and Ln to the one table set that holds both, so the kernel
    # never reloads ACT tables (set ids are positional; only values change).
    from concourse import hw_specs

    for name, funcs in hw_specs.get_activation_tables(nc.m.arch).items():
        if name != "natural_log_exp_and_others":
            funcs.discard(Act.Exp)
            funcs.discard(Act.Ln)

    # ---------------- DRAM I/O ----------------
    hsT = nc.dram_tensor("hsT", [HIDDEN, S], dt.float8e4, kind="ExternalInput")
    wqk = nc.dram_tensor("wqk", [HIDDEN, 512], dt.float8e4, kind="ExternalInput")
    wv = nc.dram_tensor("wv", [HIDDEN, 256], dt.float8e4, kind="ExternalInput")
    wd = nc.dram_tensor("wd", [256, HIDDEN], dt.float8e4, kind="ExternalInput")
    bqk = nc.dram_tensor("bqk", [256, 1], dt.float32, kind="ExternalInput")
    hs_res = nc.dram_tensor(
        "hs_res", [PAD_ROWS, HIDDEN], dt.float32, kind="ExternalInput"
    )
    out = nc.dram_tensor("out", [PAD_ROWS, HIDDEN], dt.float32, kind="ExternalOutput")

    # internal DRAM for the collective (cannot use I/O tensors); fp8 halves
    # the wire bytes -- partials are pre-scaled by 2^8 into fp8 range and
    # descaled after the reduce (CCE sums in fp32)
    cc_in = [
        nc.dram_tensor(f"cc_in{g}", [(hi - lo) * 128, HIDDEN], dt.float8e4)
        for g, (lo, hi) in enumerate(RS_CHUNKS)
    ]
    cc_out = [
        nc.dram_tensor(f"cc_out{g}", [RS_SZ[g], HIDDEN], dt.float8e4)
        for g in range(NCHUNK)
    ]

    with tile.TileContext(nc) as tc:
        with (
            tc.tile_pool(name="persist", bufs=1) as persist,
            tc.tile_pool(name="pT_pool", bufs=4) as pT_pool,
            tc.tile_pool(name="work", bufs=3) as work,
            tc.tile_pool(name="ln", bufs=2) as lnp,
            tc.tile_pool(name="psmm", bufs=3, space="PSUM") as psmm,
            tc.tile_pool(name="psctx", bufs=1, space="PSUM") as psctx,
        ):
            # ---------------- persistent SBUF loads ----------------
            zero_sb = persist.tile([128, 1], dt.float32, name="zero_sb")
            nc.vector.memset(zero_sb, 0.0)
            nc.const_aps.aps[(dt.float32, 0.0)] = zero_sb
            # the LN input x' = cc_out + 256*res is 256x the true x (LN is
            # scale-invariant), so eps scales by 256^2
            eps_sb = persist.tile([128, 1], dt.float32, name="eps_sb")
            nc.vector.memset(eps_sb, LN_EPS * SCC * SCC)
            # coalesced input DMAs; hsT split so the first QK matmuls start
            # while later chunks are in flight
            hsT_all = persist.tile([128, 8, S], dt.float8e4, name="hsT_all")
            hsT_r = hsT[:, :].rearrange("(c p) t -> p c t", p=128)
            wqk_all = persist.tile([128, 8, 512], dt.float8e4, name="wqk_all")
            nc.sync.dma_start(
                out=wqk_all, in_=wqk[:, :].rearrange("(c p) n -> p c n", p=128)
            )
            # split hsT by token-half: the prologue K0nh0/Q0nh0 matmuls only
            # touch tokens 0:1024, so PE starts ~7us earlier
            nc.sync.dma_start(
                out=hsT_all[:, :, 0:1024], in_=hsT_r[:, :, 0:1024]
            )
            nc.sync.dma_start(
                out=hsT_all[:, :, 1024:2048], in_=hsT_r[:, :, 1024:2048]
            )
            wv_all = persist.tile([128, 8, 256], dt.float8e4, name="wv_all")
            nc.sync.dma_start(
                out=wv_all, in_=wv[:, :].rearrange("(c p) n -> p c n", p=128)
            )
            bqk_all = persist.tile([128, 2], dt.float32, name="bqk_all")
            nc.sync.dma_start(
                out=bqk_all, in_=bqk[:, :].rearrange("(m p) o -> p (m o)", p=128)
            )
            # wd (needed ~90us in) and the residual (needed only at LN time)
            # load AFTER the prologue-critical hsT/wqk/wv so the first QK
            # matmuls are not DMA-gated behind 2.5MB of late-use data
            wd_all = persist.tile([128, 2, HIDDEN], dt.float8e4, name="wd_all")
            res_all = persist.tile([128, NCHUNK, HIDDEN], dt.float32, name="res_all")

            # K tiles per pair: [128 qk-dims (even head 0:64, odd 64:128), S]
            kT_sb = [
                persist.tile([128, S], dt.bfloat16, name=f"kT{p}")
                for p in range(PAIRS)
            ]
            # zero-padded Q per (pair, head): head l occupies rows l*64:(l+1)*64,
            # other 64 rows stay exactly 0 so score matmuls run the full
            # 128-contraction with K-pair tiles as the (shared) lhsT.
            # (merging both heads into one N=1024 matmul fails walrus codegen)
            qT_sb = [
                [
                    persist.tile([128, S], dt.bfloat16, name=f"qT{p}_{l}")
                    for l in range(2)
                ]
                for p in range(PAIRS)
            ]
            for p in range(PAIRS):
                for l in range(2):
                    nc.vector.memset(qT_sb[p][l], 0.0)
            # V tiles: [128 tok, kp, i(kc in pair), 4*(V_h(64)|ones(64))] fp8
            v_all = persist.tile([128, NKP, 2, 512], dt.float8e4, name="v_all")
            nc.vector.memset(
                v_all.rearrange("p a b (g c) -> p a b g c", c=128)[:, :, :, :, 64:128],
                1.0,
            )
            # ctx^T (normalized, 32x scale, fp8): dim1 = pair chunk
            ctxT_sb = persist.tile([128, PAIRS, S], dt.float8e4, name="ctxT_sb")

            # ---------------- projection emitters ----------------
            def emit_k_chunk(p, nh):
                # K pair p, token half nh -> kT_sb[p][:, nh*1024:(nh+1)*1024]
                ps = psmm.tile([128, 1024], dt.float32, name="ps_mm")
                for j in range(2):
                    span = slice(nh * 1024 + j * 512, nh * 1024 + (j + 1) * 512)
                    for cp in range(NCP):
                        nc.tensor.matmul(
                            ps[:, j * 512 : (j + 1) * 512],
                            lhsT=wqk_all[:, 2 * cp : 2 * cp + 2, p * 256 : p * 256 + 128],
                            rhs=hsT_all[:, 2 * cp : 2 * cp + 2, span],
                            start=(cp == 0),
                            stop=(cp == NCP - 1),
                            perf_mode=PM.DoubleRow,
                        )
                nc.vector.tensor_copy(
                    out=kT_sb[p][:, nh * 1024 : (nh + 1) * 1024], in_=ps
                )

            def emit_q_chunk(p, nh):
                ps = psmm.tile([128, 1024], dt.float32, name="ps_mm")
                for j in range(2):
                    span = slice(nh * 1024 + j * 512, nh * 1024 + (j + 1) * 512)
                    for cp in range(NCP):
                        nc.tensor.matmul(
                            ps[:, j * 512 : (j + 1) * 512],
                            lhsT=wqk_all[
                                :, 2 * cp : 2 * cp + 2, p * 256 + 128 : p * 256 + 256
                            ],
                            rhs=hsT_all[:, 2 * cp : 2 * cp + 2, span],
                            start=(cp == 0),
                            stop=(cp == NCP - 1),
                            perf_mode=PM.DoubleRow,
                        )
                # evac per head with Q bias into the zero-padded tiles
                for l in range(2):
                    nc.vector.tensor_scalar_add(
                        out=qT_sb[p][l][
                            l * 64 : (l + 1) * 64, nh * 1024 : (nh + 1) * 1024
                        ],
                        in0=ps[l * 64 : (l + 1) * 64, :],
                        scalar1=bqk_all[l * 64 : (l + 1) * 64, p : p + 1],
                    )

            def emit_v_chunk(t):
                # V token-chunk t -> v_all[:, t//2, t%2, g*128:g*128+64]
                ps = psmm.tile([128, 1024], dt.float32, name="ps_mm")
                for cp in range(NCP):
                    nc.tensor.matmul(
                        ps[:, 0:256],
                        lhsT=hsT_all[:, 2 * cp : 2 * cp + 2, t * 128 : (t + 1) * 128],
                        rhs=wv_all[:, 2 * cp : 2 * cp + 2, :],
                        start=(cp == 0),
                        stop=(cp == NCP - 1),
                        perf_mode=PM.DoubleRow,
                    )
                vt = v_all[:, t // 2, t % 2, :].rearrange("p (g c) -> p g c", c=128)
                nc.vector.tensor_copy(
                    out=vt[:, :, 0:64],
                    in_=ps[:, 0:256].rearrange("p (g c) -> p g c", c=64),
                )

            # prologue: just enough projection for (qt0, p0) -- K pair 0,
            # Q pair 0 first token-half, first V chunks -- so ACT starts
            # ~15us in; the rest weaves into the qt0-p0 attention PE slack
            # (2 jobs at kc 0/1 then one per kc keeps every operand at
            # least one k-pair ahead of its first use)
            emit_k_chunk(0, 0)
            emit_q_chunk(0, 0)
            emit_v_chunk(0)
            emit_v_chunk(1)
            emit_k_chunk(0, 1)
            emit_v_chunk(2)
            emit_v_chunk(3)
            nc.sync.dma_start(
                out=wd_all, in_=wd[:, :].rearrange("(c p) n -> p c n", p=128)
            )
            nc.sync.dma_start(
                out=res_all,
                in_=hs_res[:, :].rearrange("(g p) n -> p g n", p=128),
            )

            weave_jobs = []
            weave_jobs.append(lambda: emit_k_chunk(1, 0))
            weave_jobs.append(lambda: emit_k_chunk(1, 1))
            weave_jobs.append(lambda: emit_q_chunk(1, 0))
            for t in range(4, NTOK):
                weave_jobs.append(lambda t=t: emit_v_chunk(t))
            weave_jobs.append(lambda: emit_q_chunk(0, 1))
            weave_jobs.append(lambda: emit_q_chunk(1, 1))

            # ---------------- phase 2: attention + dense + RS ----------------
            cc_insts = []
            dense_state = {"last_evac": None}

            def emit_dense_ti(ti_g, evac_on_act=False):
                tok = ti_g * 128
                ps_d = psmm.tile([128, 1024], dt.float32, name="ps_mm")
                for j in range(2):
                    nc.tensor.matmul(
                        ps_d[:, j * 512 : (j + 1) * 512],
                        lhsT=ctxT_sb[:, :, tok : tok + 128],
                        rhs=wd_all[:, :, j * 512 : (j + 1) * 512],
                        start=True,
                        stop=True,
                        perf_mode=PM.DoubleRow,
                    )
                dsb = work.tile([128, 1024], dt.float8e4, name="dsb")
                if evac_on_act:
                    # tail-only: ACT is idle after the last exp; splitting the
                    # final evacuations across ACT+DVE fires RS3 sooner
                    dense_state["last_evac"] = nc.scalar.activation(
                        out=dsb, in_=ps_d, func=Act.Copy, scale=DENSE_DESCALE
                    )
                else:
                    dense_state["last_evac"] = nc.vector.tensor_scalar_mul(
                        out=dsb, in0=ps_d, scalar1=DENSE_DESCALE
                    )
                g = next(
                    i for i, (lo, hi) in enumerate(RS_CHUNKS) if lo <= ti_g < hi
                )
                lo = RS_CHUNKS[g][0]
                nc.sync.dma_start(
                    out=cc_in[g][(ti_g - lo) * 128 : (ti_g - lo + 1) * 128, :],
                    in_=dsb,
                )
                if ti_g == RS_CHUNKS[g][1] - 1:
                    cc_insts.append(
                        nc.gpsimd.collective_compute(
                            "ReduceScatter",
                            Alu.add,
                            replica_groups=REPLICA_GROUPS,
                            ins=[cc_in[g][:, :].opt()],
                            outs=[cc_out[g][:, :].opt()],
                        )
                    )

            for qt in range(NQT):
                for p in range(PAIRS):
                    ctx_ps = [
                        psctx.tile([128, 512], dt.float32, name=f"ps_ctx{l}")
                        for l in range(2)
                    ]

                    def emit_scores(kc, p=p, qt=qt):
                        ps_s = psmm.tile([128, 1024], dt.float32, name="ps_mm")
                        for l in range(2):
                            nc.tensor.matmul(
                                ps_s[:, l * 512 : (l + 1) * 512],
                                lhsT=kT_sb[p][:, kc * 128 : (kc + 1) * 128],
                                rhs=qT_sb[p][l][:, qt * 512 : (qt + 1) * 512],
                                start=True,
                                stop=True,
                            )
                        return ps_s

                    # software pipeline: scores run one k-chunk ahead so the
                    # PE never sits in-order behind ctx(kp)'s wait on exp
                    ps_s = emit_scores(0)
                    pT = None
                    ctx_pending = []
                    for kc in range(NTOK):
                        kp, i = kc // 2, kc % 2
                        if i == 0:
                            pT = pT_pool.tile(
                                [128, 2, 1024], dt.float8e4, name="pT"
                            )
                        ps_s_next = emit_scores(kc + 1) if kc + 1 < NTOK else None
                        nc.scalar.activation(
                            out=pT[:, i, :], in_=ps_s, func=Act.Exp, scale=EXP_SCALE
                        )
                        ps_s = ps_s_next

                        def emit_ctx(kp, pT, p=p, ctx_ps=ctx_ps):
                            # ctx via fp8 DoubleRow over the k-chunk pair
                            for l in range(2):
                                h = 2 * p + l
                                nc.tensor.matmul(
                                    ctx_ps[l],
                                    lhsT=v_all[
                                        :, kp, :, h * 128 : (h + 1) * 128
                                    ],
                                    rhs=pT[:, :, l * 512 : (l + 1) * 512],
                                    start=(kp == 0),
                                    stop=(kp == NKP - 1),
                                    perf_mode=PM.DoubleRow,
                                )

                        # defer the first two k-pairs' ctx to kc=5 so the
                        # in-order PE queue never blocks on the previous
                        # pair's normalize (DVE) right at the boundary
                        if i == 1:
                            if kp < 2:
                                ctx_pending.append((kp, pT))
                            else:
                                while ctx_pending:
                                    emit_ctx(*ctx_pending.pop(0))
                                emit_ctx(kp, pT)
                        # weave: remaining projections during qt0-p0, then
                        # the previous q-tile's dense matmuls
                        if p == 0 and qt == 0 and weave_jobs:
                            for _ in range(2 if kc < 2 else 1):
                                if weave_jobs:
                                    weave_jobs.pop(0)()
                        if p == 0 and qt >= 1 and kc in (3, 6, 8, 10):
                            emit_dense_ti(
                                (qt - 1) * 4 + {3: 0, 6: 1, 8: 2, 10: 3}[kc]
                            )
                    # normalize: ctx[0:64] / den[64:128] -> ctxT (fp8, 32x).
                    # The partition-shifting copy is required: the custom-DVE
                    # reciprocal cannot read partitions 64:128 while writing
                    # 0:64 (run8 regression), and tensor_tensor needs aligned
                    # partition ranges.
                    for l in range(2):
                        den_sb = work.tile([64, 512], dt.float32, name="den_sb")
                        nc.vector.tensor_copy(
                            out=den_sb, in_=ctx_ps[l][64:128, :]
                        )
                        rec = work.tile([64, 512], dt.float32, name="rec")
                        nc.vector.reciprocal_approx_fast(out=rec, in_=den_sb)
                        nc.vector.tensor_tensor(
                            out=ctxT_sb[
                                l * 64 : (l + 1) * 64,
                                p,
                                qt * 512 : (qt + 1) * 512,
                            ],
                            in0=ctx_ps[l][0:64, :],
                            in1=rec,
                            op=Alu.mult,
                        )
            # leftover weave jobs (shouldn't happen) + last q-tile's dense
            for job in weave_jobs:
                job()
            for ti in range(4):
                emit_dense_ti(12 + ti, evac_on_act=(ti % 2 == 0))
            last_evac = dense_state["last_evac"]

            # ---------------- phase 3: residual + LayerNorm ----------------
            from concourse.bass import _add_dep_helper

            for g in range(NCHUNK):
                sz = RS_SZ[g]
                xb = lnp.tile([128, HIDDEN], dt.float8e4, name="xb")
                xb_dma = nc.sync.dma_start(out=xb[:sz, :], in_=cc_out[g][:, :])
                _add_dep_helper(
                    xb_dma.ins,
                    last_evac.ins,
                    sync=True,
                    reason="LN after attention/dense (keep queues unblocked)",
                )
                # x' = cc_out(fp8, 256x scale) + 256*res in one add; the
                # 256x cancels in (x-mu)*rstd since eps is scaled 256^2
                x = lnp.tile([128, HIDDEN], dt.float32, name="x")
                nc.vector.tensor_tensor(
                    out=x[:sz, :],
                    in0=xb[:sz, :],
                    in1=res_all[:sz, g, :],
                    op=Alu.add,
                )
                stats = lnp.tile([128, 2, 6], dt.float32, name="stats")
                xv = x.rearrange("p (s f) -> p s f", f=512)
                for i in range(2):
                    nc.vector.bn_stats(out=stats[:sz, i, :], in_=xv[:sz, i, :])
                mv = lnp.tile([128, 2], dt.float32, name="mv")
                nc.vector.bn_aggr(out=mv[:sz, :], in_=stats[:sz, :, :])
                # rstd = exp(-0.5 * ln(var + eps)) -- stays in the exp/ln table set
                lnv = lnp.tile([128, 1], dt.float32, name="lnv")
                nc.scalar.activation(
                    out=lnv[:sz, :], in_=mv[:sz, 1:2], func=Act.Ln, bias=eps_sb[:sz, :]
                )
                rstd = lnp.tile([128, 1], dt.float32, name="rstd")
                nc.scalar.activation(
                    out=rstd[:sz, :], in_=lnv[:sz, :], func=Act.Exp, scale=-0.5
                )
                y = lnp.tile([128, HIDDEN], dt.float32, name="y")
                nc.vector.tensor_scalar(
                    out=y[:sz, :],
                    in0=x[:sz, :],
                    scalar1=mv[:sz, 0:1],
                    scalar2=rstd[:sz, :],
                    op0=Alu.subtract,
                    op1=Alu.mult,
                )
                nc.sync.dma_start(
                    out=out[g * 128 : g * 128 + sz, :], in_=y[:sz, :]
                )

    nc.compile()
    return nc


_PROGRAM = None


def _get_program():
    global _PROGRAM
    if _PROGRAM is None:
        _PROGRAM = _build_program()
    return _PROGRAM


def _prep_core_inputs(hidden_states, w_qkv, b_qkv, w_dense, b_dense):
    """Build the 8 per-core input maps (numpy, host-side sharding)."""
    hs = np.asarray(hidden_states, dtype=np.float32)
    w_qkv = np.asarray(w_qkv, dtype=np.float32)
    b_qkv = np.asarray(b_qkv, dtype=np.float32)
    w_dense = np.asarray(w_dense, dtype=np.float32)
    b_dense = np.asarray(b_dense, dtype=np.float32)

    # v-channel bias folded into a host-side output bias:
    # b_out = b_dense + b_v_full @ w_dense   (b_v in ctx channel order)
    bv_full = np.empty((HIDDEN,), dtype=np.float64)
    for g in range(HEADS):
        bv_full[g * HD : (g + 1) * HD] = b_qkv[g * 192 + 128 : g * 192 + 192]
    # w_dense rows are already in (head, d) = g*64+d order, matching bv_full
    b_out = (
        b_dense.astype(np.float64)
        + bv_full @ w_dense.astype(np.float64)
    ).astype(np.float32)

    in_maps = []
    for r in range(N_CORES):
        b = r // TP
        tp = r % TP
        gheads = [4 * tp + l for l in range(LHEADS)]

        hsT_f8 = np.ascontiguousarray(hs[b].T).astype(FP8)  # [1024, 2048]

        # wqk column order: per pair: K(even) K(odd) (128) then Q(even) Q(odd)
        wqk_cols = np.empty((HIDDEN, 512), dtype=np.float32)
        bq_vec = np.empty((256,), dtype=np.float32)
        for p in range(PAIRS):
            for l in range(2):
                g = gheads[2 * p + l]
                kcol = slice(g * 192 + 64, g * 192 + 128)
                qcol = slice(g * 192, g * 192 + 64)
                base = p * 256
                wqk_cols[:, base + l * 64 : base + (l + 1) * 64] = w_qkv[:, kcol]
                wqk_cols[:, base + 128 + l * 64 : base + 128 + (l + 1) * 64] = w_qkv[
                    :, qcol
                ]
                bq_vec[p * 128 + l * 64 : p * 128 + (l + 1) * 64] = b_qkv[qcol]

        wv_cols = np.empty((HIDDEN, 256), dtype=np.float32)
        for l, g in enumerate(gheads):
            wv_cols[:, l * 64 : (l + 1) * 64] = w_qkv[
                :, g * 192 + 128 : g * 192 + 192
            ]

        wd_rows = np.empty((256, HIDDEN), dtype=np.float32)
        for l, g in enumerate(gheads):
            wd_rows[l * 64 : (l + 1) * 64, :] = w_dense[g * 64 : (g + 1) * 64, :]

        # residual shard (+ folded output bias); padded layout: chunk g's
        # sz rows live at [g*128, g*128+sz), covering global tokens
        # lo*128 + tp*sz + [0, sz)
        # residual pre-scaled by SCC so the fp8 collective output adds
        # directly (LN is scale-invariant; eps is scaled SCC^2 in-kernel)
        res = np.zeros((PAD_ROWS, HIDDEN), dtype=np.float32)
        for g, (lo, hi) in enumerate(RS_CHUNKS):
            sz = RS_SZ[g]
            t0 = lo * 128 + tp * sz
            res[g * 128 : g * 128 + sz, :] = (
                hs[b, t0 : t0 + sz, :] + b_out
            ) * SCC

        in_maps.append(
            {
                "hsT": hsT_f8,
                "wqk": (wqk_cols * SW).astype(FP8),
                "wv": (wv_cols * SVS).astype(FP8),
                "wd": (wd_rows * SD).astype(FP8),
                "bqk": (bq_vec * SW).reshape(256, 1),
                "hs_res": res,
            }
        )
    return in_maps


def kernel(hidden_states, w_qkv, b_qkv, w_dense, b_dense, ln_gamma, ln_beta,
           _return_perf=False, **run_kwargs):
    ln_gamma = np.asarray(ln_gamma, dtype=np.float32)
    ln_beta = np.asarray(ln_beta, dtype=np.float32)
    gamma_one = np.allclose(ln_gamma, 1.0)
    beta_zero = np.allclose(ln_beta, 0.0)

    nc = _get_program()
    in_maps = _prep_core_inputs(hidden_states, w_qkv, b_qkv, w_dense, b_dense)
    res = run_bass_kernel_spmd(
        nc, in_maps, core_ids=list(range(N_CORES)), **run_kwargs
    )

    full = np.empty((B, S, HIDDEN), dtype=np.float32)
    for r in range(N_CORES):
        b = r // TP
        tp = r % TP
        o = res.results[r]["out"]
        for g, (lo, hi) in enumerate(RS_CHUNKS):
            sz = RS_SZ[g]
            t0 = lo * 128 + tp * sz
            full[b, t0 : t0 + sz, :] = o[g * 128 : g * 128 + sz, :]

    if not (gamma_one and beta_zero):
        # spec fills gamma=ones, beta=zeros; fall back on host if they differ
        full = full * ln_gamma[None, None, :] + ln_beta[None, None, :]

    if _return_perf:
        return full, res
    return full
